# revision 24
# baseline (speedup 1.0000x reference)
"""Trainium2 Bass kernel for windowed embedding lookup (nn_AttentionLayer).

Computation:
  out[b,s,e] = sum_k w[k,e] * data[snip_b, clip(inputs[b,s]+k-5, 0, 165), 0, e]

Strategy (data-parallel over batch, 2 batches per core on 8 cores):
  1. Host stages the table as bf16 with the clip-padding baked in
     ([100*128, 6*176]: per snippet, e-major chunks of 176 padded
     positions), so each batch's slice is ONE contiguous HWDGE dynamic
     DMA (snippet id in a sync/scalar-engine register via values_load).
  2. Diagonal weight matrices diag(w[k, e-chunk]) are built on-device
     from a tiny [128, 66] staged weight tile via affine_select
     (broadcast-read + off-diagonal fill 0) on DVE/GPSIMD.
  3. 11-tap clip-padded convolution C[p,e] = sum_k w[k,e]*T[p+k-5,e]
     on TensorE as PSUM-accumulated matmuls (lhsT = shifted T window,
     rhs = diag), 2 row-blocks x 6 chunks x 11 taps per batch.
  4. Row gather out[s] = C[inputs[s]] as one-hot matmuls (iota +
     is_equal one-hots, 2 row-blocks accumulated in PSUM).
  5. PSUM drained to SBUF on DVE/ACT/GPSIMD round-robin; per-tile
     output DMAs issued alternately from sync/scalar HWDGE queues.
  A few warm-up matmuls run during the DMA preamble so the PE p-state
  is ramped when the real work arrives.
"""

import sys

for _p in ("/opt/trn_rl_repo",):
    if _p not in sys.path:
        sys.path.insert(0, _p)

import numpy as np

N_CORES = 8
B = 16
BPC = B // N_CORES  # batches per core
S = 1126
E = 768
EC = 6  # number of 128-wide e chunks
P = 166  # table positions
PPAD = 176  # padded positions (5 on each side)
W = 11
NSNIP = 100
MTILES = (S + 127) // 128  # 9
WARM_MMS = 6

_cache = {}


def _build():
    import concourse.bass as bass
    import concourse.mybir as mybir
    import concourse.tile as tile
    from concourse import bacc

    f32 = mybir.dt.float32
    bf16 = mybir.dt.bfloat16
    i32 = mybir.dt.int32
    AOT = mybir.AluOpType
    ET = mybir.EngineType

    nc = bacc.Bacc()

    snips_d = nc.declare_dram_parameter("snips", [1, BPC], i32, isOutput=False)
    inps_d = nc.declare_dram_parameter(
        "inps", [1, BPC * S], bf16, isOutput=False
    )
    # row (snip*128 + i) holds [c*176 + j] -> data[snip, clip(j-5), 0, c*128+i]
    dataT2p = nc.declare_dram_parameter(
        "dataT2p", [NSNIP * 128, EC * PPAD], bf16, isOutput=False
    )
    # diagonal weight matrices: [i, (c*11+k)*128 + j] = w[k, c*128+i] iff i==j
    diagw = nc.declare_dram_parameter(
        "diagw", [128, EC * W * 128], bf16, isOutput=False
    )
    # bf16 output, widened to f32 on the host
    out = nc.declare_dram_parameter("out", [BPC * S, E], bf16, isOutput=True)

    with tile.TileContext(nc) as tc:
        with (
            tc.tile_pool(name="const", bufs=1) as constp,
            tc.tile_pool(name="work", bufs=1) as workp,
            tc.tile_pool(name="ob", bufs=8) as obp,
            tc.tile_pool(name="psA", bufs=2, space="PSUM") as psA,
            tc.tile_pool(name="psB", bufs=2, space="PSUM") as psB,
        ):
            # ---------- tiny constants ----------
            ones1 = constp.tile([1, 128], bf16)
            nc.vector.memset(ones1[:], 1.0)
            warm = constp.tile([128, 512], bf16)
            nc.vector.memset(warm[:], 0.001)

            iota_i = constp.tile([128, 1], i32)
            nc.gpsimd.iota(iota_i[:], [[1, 1]], base=0, channel_multiplier=1)
            iota_f = constp.tile([128, 1], f32)
            nc.vector.tensor_copy(iota_f[:], iota_i[:])
            iota_f_hi = constp.tile([128, 1], f32)
            nc.vector.tensor_scalar_add(iota_f_hi[:], iota_f[:], 128.0)

            # ---------- input DMAs (issue ASAP, spread across queues) ----
            # t2 slices go first (they gate conv); the 2.16MB diagw
            # streams behind them, chunk 1 first to match conv's
            # consumption order 1,0,2,3,4,5.
            snipt = workp.tile([1, BPC], i32, tag="snipt")
            nc.sync.dma_start(out=snipt[:], in_=snips_d[:])
            inprt = workp.tile([1, BPC * S], bf16, tag="inprt")
            nc.scalar.dma_start(out=inprt[:], in_=inps_d[:])

            diagb = constp.tile([128, EC * W, 128], bf16)

            def diag_chunk(c, eng):
                eng.dma_start(
                    out=diagb[:, c * W : (c + 1) * W, :],
                    in_=diagw[:, c * W * 128 : (c + 1) * W * 128].rearrange(
                        "p (k j) -> p k j", j=128
                    ),
                )

            snip_v = [
                nc.values_load(
                    snipt[0:1, 0:1],
                    engines=[ET.SP],
                    min_val=0,
                    max_val=NSNIP - 1,
                    skip_runtime_bounds_check=True,
                ),
                nc.values_load(
                    snipt[0:1, 1:2],
                    engines=[ET.Activation],
                    min_val=0,
                    max_val=NSNIP - 1,
                    skip_runtime_bounds_check=True,
                ),
            ]
            t2 = []
            for b, eng in ((0, nc.sync), (1, nc.scalar)):
                t2b = workp.tile([128, EC, PPAD], bf16, tag=f"t2_{b}")
                eng.dma_start(
                    out=t2b[:, :, :],
                    in_=dataT2p[bass.ts(snip_v[b], 128), :].rearrange(
                        "p (c j) -> p c j", j=PPAD
                    ),
                )
                t2.append(t2b)
            diag_chunk(1, nc.sync)
            diag_chunk(0, nc.scalar)
            diag_chunk(2, nc.sync)
            diag_chunk(3, nc.scalar)
            diag_chunk(4, nc.sync)
            diag_chunk(5, nc.scalar)

            # ---------- PE warm-up (ramp the p-state) ------------------
            warm_ps = psB.tile([128, E], f32, tag="go")
            for wi in range(WARM_MMS):
                nc.tensor.matmul(
                    out=warm_ps[:, 0:512],
                    lhsT=warm[:, 0:128],
                    rhs=warm[:, 0:512],
                    start=(wi == 0),
                    stop=(wi == WARM_MMS - 1),
                )
            warm_close = constp.tile([128, 1], f32)
            nc.vector.tensor_copy(warm_close[:], warm_ps[:, 0:1])

            # ---------- input broadcast + one-hots ---------------------
            # inpb[b][p, s] = inputs[b, s] replicated over 128 partitions
            inpb = []
            chunks = [(0, 512), (512, 512), (1024, S - 1024)]
            for b in range(BPC):
                ib = workp.tile([128, S], bf16, tag=f"inpb{b}")
                for ci, (n0, nw) in enumerate(chunks):
                    ps_in = psB.tile([128, E], f32, tag="go")
                    nc.tensor.matmul(
                        out=ps_in[:, :nw],
                        lhsT=ones1[:, :],
                        rhs=inprt[0:1, b * S + n0 : b * S + n0 + nw],
                        start=True,
                        stop=True,
                    )
                    nc.vector.tensor_copy(ib[:, n0 : n0 + nw], ps_in[:, :nw])
                inpb.append(ib)

            oh = []
            for b in range(BPC):
                oh0 = workp.tile([128, S], bf16, tag=f"oh0_{b}")
                oh1 = workp.tile([128, S], bf16, tag=f"oh1_{b}")
                nc.vector.tensor_scalar(
                    oh0[:], inpb[b][:], iota_f[:, :1], None, AOT.is_equal
                )
                nc.vector.tensor_scalar(
                    oh1[:], inpb[b][:], iota_f_hi[:, :1], None, AOT.is_equal
                )
                oh.append((oh0, oh1))

            # ---------- per batch: conv on PE, then gather + store -----
            # GPSIMD cannot touch PSUM: drains alternate DVE / ACT only.
            def drain(idx, dst, src):
                if idx % 2 == 0:
                    nc.vector.tensor_copy(dst, src)
                else:
                    nc.scalar.copy(dst, src)

            CORDER = (1, 0, 2, 3, 4, 5)  # matches diag chunk DMA arrival
            groups = [(0, 3), (3, 3), (6, 2), (8, 1)]
            for b in range(BPC):
                # conv: C = sum_k diag(w_k) @ shifted window
                ccat = []
                for blk in range(2):
                    mw = 128 if blk == 0 else P - 128
                    psc = psA.tile([128, E], f32, tag="cv")
                    for c in CORDER:
                        for k in range(W):
                            nc.tensor.matmul(
                                out=psc[:mw, c * 128 : (c + 1) * 128],
                                lhsT=t2[b][:, c, blk * 128 + k : blk * 128 + k + mw],
                                rhs=diagb[:, c * W + k, :],
                                start=(k == 0),
                                stop=(k == W - 1),
                            )
                    cc = workp.tile([128, E], bf16, tag=f"cc{b}_{blk}")
                    drain(b * 2 + blk, cc[:mw, :], psc[:mw, :])
                    ccat.append(cc)

                # gather + grouped output DMAs: m-tiles (0,1,2) (3,4,5)
                # (6,7) (8) become 4 strided DMAs per batch.
                cc0, cc1 = ccat
                oh0, oh1 = oh[b]
                for gi, (m0, gn) in enumerate(groups):
                    ob = obp.tile([128, 3, E], bf16, tag="ob")
                    for g in range(gn):
                        m = m0 + g
                        mw = min(128, S - m * 128)
                        pso = psB.tile([128, E], f32, tag="go")
                        for ohx, ccx, st in ((oh0, cc0, True), (oh1, cc1, False)):
                            for n0, nw in ((0, 512), (512, 256)):
                                nc.tensor.matmul(
                                    out=pso[:mw, n0 : n0 + nw],
                                    lhsT=ohx[:, m * 128 : m * 128 + mw],
                                    rhs=ccx[:, n0 : n0 + nw],
                                    start=st,
                                    stop=not st,
                                )
                        drain(b * MTILES + m, ob[:mw, g, :], pso[:mw, :])
                    r0 = b * S + m0 * 128
                    nrows = min(gn * 128, S - m0 * 128)
                    dma_eng = nc.sync if (b * 4 + gi) % 2 == 0 else nc.scalar
                    if nrows == gn * 128:
                        dma_eng.dma_start(
                            out=out[r0 : r0 + nrows, :].rearrange(
                                "(g p) e -> p g e", p=128
                            ),
                            in_=ob[:, 0:gn, :],
                        )
                    else:
                        dma_eng.dma_start(
                            out=out[r0 : r0 + nrows, :],
                            in_=ob[:nrows, 0, :],
                        )

    nc.finalize()
    return nc


def _get_nc():
    if "nc" not in _cache:
        _cache["nc"] = _build()
    return _cache["nc"]


def _prep_shared(data, w):
    # layout-only host staging (transpose/reshape/edge-pad/dtype-cast)
    import ml_dtypes

    d0 = np.asarray(data, dtype=np.float32)[:, :, 0, :]  # [100, 166, 768]
    pos = np.clip(np.arange(PPAD) - 5, 0, P - 1)
    dp = d0[:, pos, :]  # [100, 176, 768] with clip-pads baked in
    dp = np.transpose(dp, (0, 2, 1))  # [100, 768, 176]
    dp = dp.reshape(NSNIP, EC, 128, PPAD).transpose(0, 2, 1, 3)
    dataT2p = np.ascontiguousarray(
        dp.reshape(NSNIP * 128, EC * PPAD).astype(ml_dtypes.bfloat16)
    )
    wT = np.asarray(w, dtype=np.float32).T  # [768, 11]
    w2 = wT.reshape(EC, 128, W).transpose(1, 0, 2)  # [128, EC, W]
    diagw = np.zeros((128, EC * W, 128), dtype=ml_dtypes.bfloat16)
    ii = np.arange(128)
    diagw[ii, :, ii] = w2.reshape(128, EC * W).astype(ml_dtypes.bfloat16)
    diagw = np.ascontiguousarray(diagw.reshape(128, EC * W * 128))
    return dataT2p, diagw


def kernel(inputs, code_snippet_id, data, w, _trace=False):
    import ml_dtypes
    from concourse.bass_utils import run_bass_kernel_spmd

    nc = _get_nc()
    inputs = np.asarray(inputs, dtype=np.int32)
    code_snippet_id = np.asarray(code_snippet_id, dtype=np.int32)
    dataT2p, diagw = _prep_shared(data, w)

    in_maps = []
    for ci in range(N_CORES):
        b0 = ci * BPC
        in_maps.append(
            {
                "snips": np.ascontiguousarray(
                    code_snippet_id[b0 : b0 + BPC].reshape(1, BPC)
                ),
                "inps": np.ascontiguousarray(
                    inputs[b0 : b0 + BPC]
                    .reshape(1, BPC * S)
                    .astype(ml_dtypes.bfloat16)
                ),
                "dataT2p": dataT2p,
                "diagw": diagw,
            }
        )

    res = run_bass_kernel_spmd(
        nc, in_maps, core_ids=list(range(N_CORES)), trace=_trace
    )
    _cache["last_results"] = res
    out = np.concatenate(
        [
            np.asarray(res.results[i]["out"]).reshape(BPC, S, E)
            for i in range(N_CORES)
        ],
        axis=0,
    ).astype(np.float32)
    return out


# revision 27
# speedup vs baseline: 1.1394x; 1.1394x over previous
"""Trainium2 Bass kernel for windowed embedding lookup (nn_AttentionLayer).

Computation:
  out[b,s,e] = sum_k w[k,e] * data[snip_b, clip(inputs[b,s]+k-5, 0, 165), 0, e]

Strategy (data-parallel over batch, 2 batches per core on 8 cores):
  1. Host stages the table as bf16 with the clip-padding baked in
     ([100*128, 6*176]: per snippet, e-major chunks of 176 padded
     positions), so each batch's slice is ONE contiguous HWDGE dynamic
     DMA (snippet id in a sync/scalar-engine register via values_load).
  2. Diagonal weight matrices diag(w[k, e-chunk]) are built on-device
     from a tiny [128, 66] staged weight tile via affine_select
     (broadcast-read + off-diagonal fill 0) on DVE/GPSIMD.
  3. 11-tap clip-padded convolution C[p,e] = sum_k w[k,e]*T[p+k-5,e]
     on TensorE as PSUM-accumulated matmuls (lhsT = shifted T window,
     rhs = diag), 2 row-blocks x 6 chunks x 11 taps per batch.
  4. Row gather out[s] = C[inputs[s]] as one-hot matmuls (iota +
     is_equal one-hots, 2 row-blocks accumulated in PSUM).
  5. PSUM drained to SBUF on DVE/ACT/GPSIMD round-robin; per-tile
     output DMAs issued alternately from sync/scalar HWDGE queues.
  A few warm-up matmuls run during the DMA preamble so the PE p-state
  is ramped when the real work arrives.
"""

import sys

for _p in ("/opt/trn_rl_repo",):
    if _p not in sys.path:
        sys.path.insert(0, _p)

import numpy as np

N_CORES = 8
B = 16
BPC = B // N_CORES  # batches per core
S = 1126
E = 768
EC = 6  # number of 128-wide e chunks
P = 166  # table positions
PPAD = 176  # padded positions (5 on each side)
W = 11
NSNIP = 100
MTILES = (S + 127) // 128  # 9
WARM_MMS = 6

_cache = {}


def _build():
    import concourse.bass as bass
    import concourse.mybir as mybir
    import concourse.tile as tile
    from concourse import bacc

    f32 = mybir.dt.float32
    bf16 = mybir.dt.bfloat16
    i32 = mybir.dt.int32
    AOT = mybir.AluOpType
    ET = mybir.EngineType

    nc = bacc.Bacc()

    snips_d = nc.declare_dram_parameter("snips", [1, BPC], i32, isOutput=False)
    inps_d = nc.declare_dram_parameter(
        "inps", [1, BPC * S], bf16, isOutput=False
    )
    # row (snip*128 + i) holds [c*176 + j] -> data[snip, clip(j-5), 0, c*128+i]
    dataT2p = nc.declare_dram_parameter(
        "dataT2p", [NSNIP * 128, EC * PPAD], bf16, isOutput=False
    )
    # diagonal weight matrices: [i, (c*11+k)*128 + j] = w[k, c*128+i] iff i==j
    diagw = nc.declare_dram_parameter(
        "diagw", [128, EC * W * 128], bf16, isOutput=False
    )
    # bf16 output, widened to f32 on the host
    out = nc.declare_dram_parameter("out", [BPC * S, E], bf16, isOutput=True)

    with tile.TileContext(nc) as tc:
        with (
            tc.tile_pool(name="const", bufs=1) as constp,
            tc.tile_pool(name="work", bufs=1) as workp,
            tc.tile_pool(name="ob", bufs=8) as obp,
            tc.tile_pool(name="psA", bufs=2, space="PSUM") as psA,
            tc.tile_pool(name="psB", bufs=2, space="PSUM") as psB,
        ):
            # ---------- tiny constants ----------
            ones1 = constp.tile([1, 128], bf16)
            nc.vector.memset(ones1[:], 1.0)
            warm = constp.tile([128, 512], bf16)
            nc.vector.memset(warm[:], 0.001)

            iota_i = constp.tile([128, 1], i32)
            nc.gpsimd.iota(iota_i[:], [[1, 1]], base=0, channel_multiplier=1)
            iota_f = constp.tile([128, 1], f32)
            nc.vector.tensor_copy(iota_f[:], iota_i[:])
            iota_f_hi = constp.tile([128, 1], f32)
            nc.vector.tensor_scalar_add(iota_f_hi[:], iota_f[:], 128.0)
            iota_m48 = constp.tile([128, 1], f32)
            nc.vector.tensor_scalar_add(iota_m48[:], iota_f[:], -48.0)
            iota_p80 = constp.tile([128, 1], f32)
            nc.vector.tensor_scalar_add(iota_p80[:], iota_f[:], 80.0)

            # ---------- input DMAs (issue ASAP, spread across queues) ----
            # t2 slices go first (they gate conv); the 2.16MB diagw
            # streams behind them, chunk 1 first to match conv's
            # consumption order 1,0,2,3,4,5.
            snipt = workp.tile([1, BPC], i32, tag="snipt")
            nc.sync.dma_start(out=snipt[:], in_=snips_d[:])
            inprt = workp.tile([1, BPC * S], bf16, tag="inprt")
            nc.scalar.dma_start(out=inprt[:], in_=inps_d[:])

            diagb = constp.tile([128, EC * W, 128], bf16)

            def diag_chunk(c, eng):
                eng.dma_start(
                    out=diagb[:, c * W : (c + 1) * W, :],
                    in_=diagw[:, c * W * 128 : (c + 1) * W * 128].rearrange(
                        "p (k j) -> p k j", j=128
                    ),
                )

            snip_v = [
                nc.values_load(
                    snipt[0:1, 0:1],
                    engines=[ET.SP],
                    min_val=0,
                    max_val=NSNIP - 1,
                    skip_runtime_bounds_check=True,
                ),
                nc.values_load(
                    snipt[0:1, 1:2],
                    engines=[ET.Activation],
                    min_val=0,
                    max_val=NSNIP - 1,
                    skip_runtime_bounds_check=True,
                ),
            ]
            t2 = []
            for b, eng in ((0, nc.sync), (1, nc.scalar)):
                t2b = workp.tile([128, EC, PPAD], bf16, tag=f"t2_{b}")
                eng.dma_start(
                    out=t2b[:, :, :],
                    in_=dataT2p[bass.ts(snip_v[b], 128), :].rearrange(
                        "p (c j) -> p c j", j=PPAD
                    ),
                )
                t2.append(t2b)
            diag_chunk(1, nc.sync)
            diag_chunk(0, nc.scalar)
            diag_chunk(2, nc.sync)
            diag_chunk(3, nc.scalar)
            diag_chunk(4, nc.sync)
            diag_chunk(5, nc.scalar)

            # ---------- PE warm-up (ramp the p-state) ------------------
            warm_ps = psB.tile([128, E], f32, tag="go")
            for wi in range(WARM_MMS):
                nc.tensor.matmul(
                    out=warm_ps[:, 0:512],
                    lhsT=warm[:, 0:128],
                    rhs=warm[:, 0:512],
                    start=(wi == 0),
                    stop=(wi == WARM_MMS - 1),
                )
            warm_close = constp.tile([128, 1], f32)
            nc.vector.tensor_copy(warm_close[:], warm_ps[:, 0:1])

            # ---------- input broadcast + one-hots ---------------------
            # inpb[b][p, s] = inputs[b, s] replicated over 128 partitions
            inpb = []
            chunks = [(0, 512), (512, 512), (1024, S - 1024)]
            for b in range(BPC):
                ib = workp.tile([128, S], bf16, tag=f"inpb{b}")
                for ci, (n0, nw) in enumerate(chunks):
                    ps_in = psB.tile([128, E], f32, tag="go")
                    nc.tensor.matmul(
                        out=ps_in[:, :nw],
                        lhsT=ones1[:, :],
                        rhs=inprt[0:1, b * S + n0 : b * S + n0 + nw],
                        start=True,
                        stop=True,
                    )
                    nc.vector.tensor_copy(ib[:, n0 : n0 + nw], ps_in[:, :nw])
                inpb.append(ib)

            # one-hots for the 3 merged C row-blocks:
            # b0 rows live in blocks 0,1 (iota, iota+128);
            # b1 rows live in blocks 1,2 at offsets -48, +80.
            oh = []
            for b, (scA, scB) in enumerate(
                ((iota_f, iota_f_hi), (iota_m48, iota_p80))
            ):
                ohA = workp.tile([128, S], bf16, tag=f"ohA_{b}")
                ohB = workp.tile([128, S], bf16, tag=f"ohB_{b}")
                nc.vector.tensor_scalar(
                    ohA[:], inpb[b][:], scA[:, :1], None, AOT.is_equal
                )
                nc.vector.tensor_scalar(
                    ohB[:], inpb[b][:], scB[:, :1], None, AOT.is_equal
                )
                oh.append((ohA, ohB))

            # merged middle window: global positions 128..271
            # (48 cols of b0's tail, 96 cols of b1's head)
            t2mid = workp.tile([128, EC, 144], bf16, tag="t2mid")
            nc.vector.tensor_copy(t2mid[:, :, 0:48], t2[0][:, :, 128:176])
            nc.vector.tensor_copy(t2mid[:, :, 48:144], t2[1][:, :, 0:96])

            # ---------- conv (3 merged row-blocks) + gather + store ----
            # GPSIMD cannot touch PSUM: drains alternate DVE / ACT only.
            def drain(idx, dst, src):
                if idx % 2 == 0:
                    nc.vector.tensor_copy(dst, src)
                else:
                    nc.scalar.copy(dst, src)

            CORDER = (1, 0, 2, 3, 4, 5)  # matches diag chunk DMA arrival

            def conv_block(idx, src_fn, mw):
                psc = psA.tile([128, E], f32, tag="cv")
                for c in CORDER:
                    for k in range(W):
                        nc.tensor.matmul(
                            out=psc[:mw, c * 128 : (c + 1) * 128],
                            lhsT=src_fn(c, k, mw),
                            rhs=diagb[:, c * W + k, :],
                            start=(k == 0),
                            stop=(k == W - 1),
                        )
                cc = workp.tile([128, E], bf16, tag=f"ccb{idx}")
                drain(idx, cc[:mw, :], psc[:mw, :])
                return cc

            def gather_store(b, ohA, ccA, ohB, ccB):
                for m in range(MTILES):
                    mw = min(128, S - m * 128)
                    pso = psB.tile([128, E], f32, tag="go")
                    for ohx, ccx, st in ((ohA, ccA, True), (ohB, ccB, False)):
                        for n0, nw in ((0, 512), (512, 256)):
                            nc.tensor.matmul(
                                out=pso[:mw, n0 : n0 + nw],
                                lhsT=ohx[:, m * 128 : m * 128 + mw],
                                rhs=ccx[:, n0 : n0 + nw],
                                start=st,
                                stop=not st,
                            )
                    t = b * MTILES + m
                    ob = obp.tile([128, E], bf16, tag="ob")
                    drain(t, ob[:mw, :], pso[:mw, :])
                    dma_eng = nc.sync if t % 2 == 0 else nc.scalar
                    dma_eng.dma_start(
                        out=out[b * S + m * 128 : b * S + m * 128 + mw, :],
                        in_=ob[:mw, :],
                    )

            # block 0: b0 rows 0..127; block 1 (t2mid): b0 128..165 +
            # b1 0..79; block 2: b1 rows 80..165
            C0 = conv_block(0, lambda c, k, mw: t2[0][:, c, k : k + mw], 128)
            C1 = conv_block(1, lambda c, k, mw: t2mid[:, c, k : k + mw], 128)
            gather_store(0, oh[0][0], C0, oh[0][1], C1)
            C2 = conv_block(
                2, lambda c, k, mw: t2[1][:, c, 80 + k : 80 + k + mw], 86
            )
            gather_store(1, oh[1][0], C1, oh[1][1], C2)

    nc.finalize()
    return nc


def _get_nc():
    if "nc" not in _cache:
        _cache["nc"] = _build()
    return _cache["nc"]


def _prep_shared(data, w):
    # layout-only host staging (transpose/reshape/edge-pad/dtype-cast)
    import ml_dtypes

    d0 = np.asarray(data, dtype=np.float32)[:, :, 0, :]  # [100, 166, 768]
    pos = np.clip(np.arange(PPAD) - 5, 0, P - 1)
    dp = d0[:, pos, :]  # [100, 176, 768] with clip-pads baked in
    dp = np.transpose(dp, (0, 2, 1))  # [100, 768, 176]
    dp = dp.reshape(NSNIP, EC, 128, PPAD).transpose(0, 2, 1, 3)
    dataT2p = np.ascontiguousarray(
        dp.reshape(NSNIP * 128, EC * PPAD).astype(ml_dtypes.bfloat16)
    )
    wT = np.asarray(w, dtype=np.float32).T  # [768, 11]
    w2 = wT.reshape(EC, 128, W).transpose(1, 0, 2)  # [128, EC, W]
    diagw = np.zeros((128, EC * W, 128), dtype=ml_dtypes.bfloat16)
    ii = np.arange(128)
    diagw[ii, :, ii] = w2.reshape(128, EC * W).astype(ml_dtypes.bfloat16)
    diagw = np.ascontiguousarray(diagw.reshape(128, EC * W * 128))
    return dataT2p, diagw


def kernel(inputs, code_snippet_id, data, w, _trace=False):
    import ml_dtypes
    from concourse.bass_utils import run_bass_kernel_spmd

    nc = _get_nc()
    inputs = np.asarray(inputs, dtype=np.int32)
    code_snippet_id = np.asarray(code_snippet_id, dtype=np.int32)
    dataT2p, diagw = _prep_shared(data, w)

    in_maps = []
    for ci in range(N_CORES):
        b0 = ci * BPC
        in_maps.append(
            {
                "snips": np.ascontiguousarray(
                    code_snippet_id[b0 : b0 + BPC].reshape(1, BPC)
                ),
                "inps": np.ascontiguousarray(
                    inputs[b0 : b0 + BPC]
                    .reshape(1, BPC * S)
                    .astype(ml_dtypes.bfloat16)
                ),
                "dataT2p": dataT2p,
                "diagw": diagw,
            }
        )

    res = run_bass_kernel_spmd(
        nc, in_maps, core_ids=list(range(N_CORES)), trace=_trace
    )
    _cache["last_results"] = res
    out = np.concatenate(
        [
            np.asarray(res.results[i]["out"]).reshape(BPC, S, E)
            for i in range(N_CORES)
        ],
        axis=0,
    ).astype(np.float32)
    return out


# revision 30
# speedup vs baseline: 1.1496x; 1.0089x over previous
"""Trainium2 Bass kernel for windowed embedding lookup (nn_AttentionLayer).

Computation:
  out[b,s,e] = sum_k w[k,e] * data[snip_b, clip(inputs[b,s]+k-5, 0, 165), 0, e]

Strategy (data-parallel over batch, 2 batches per core on 8 cores):
  1. Host stages the table as bf16 with the clip-padding baked in
     ([100*128, 6*176]: per snippet, e-major chunks of 176 padded
     positions), so each batch's slice is ONE contiguous HWDGE dynamic
     DMA (snippet id in a sync/scalar-engine register via values_load);
     host also stages the diag(w) matrices (bf16, 2.2MB) which stream
     behind the t2 slices on both HWDGE queues.
  2. 11-tap clip-padded convolution C[p,e] = sum_k w[k,e]*T[p+k-5,e]
     on TensorE as PSUM-accumulated matmuls (lhsT = shifted T window,
     rhs = diag). The two batches' C tables (2x166 rows) are merged
     into THREE 128-row blocks (block 1 spans b0's tail + b1's head
     via a small DVE-merged window tile), saving 66 matmuls.
  3. Row gather out[s] = C[inputs[s]] as one-hot matmuls (iota +
     is_equal one-hots at offsets 0/+128 for b0 and -48/+80 for b1,
     2 row-blocks accumulated in PSUM per output tile).
  4. PSUM gather tiles drain split DVE|ACT (half columns each) into
     bf16 SBUF; per-tile output DMAs alternate sync/scalar HWDGE
     queues; the host widens the bf16 output to f32.
  5. PE emission order conv-b0 -> gather-b0 -> conv-blk2 -> gather-b1
     starts the output stream ~13us earlier; warm-up matmuls during
     the DMA preamble keep the PE p-state ramped (2.4GHz).
Measured: ~54-59us HW exec for the full 8-core SPMD NEFF (baseline
was ~64us; ~7us of that is fixed Tile preamble and ~3us teardown),
rel err ~2.9e-3 (bf16 table/one-hot/output quantization).
"""

import sys

for _p in ("/opt/trn_rl_repo",):
    if _p not in sys.path:
        sys.path.insert(0, _p)

import numpy as np

N_CORES = 8
B = 16
BPC = B // N_CORES  # batches per core
S = 1126
E = 768
EC = 6  # number of 128-wide e chunks
P = 166  # table positions
PPAD = 176  # padded positions (5 on each side)
W = 11
NSNIP = 100
MTILES = (S + 127) // 128  # 9
WARM_MMS = 6

_cache = {}


def _build():
    import concourse.bass as bass
    import concourse.mybir as mybir
    import concourse.tile as tile
    from concourse import bacc

    f32 = mybir.dt.float32
    bf16 = mybir.dt.bfloat16
    i32 = mybir.dt.int32
    AOT = mybir.AluOpType
    ET = mybir.EngineType

    nc = bacc.Bacc()

    snips_d = nc.declare_dram_parameter("snips", [1, BPC], i32, isOutput=False)
    inps_d = nc.declare_dram_parameter(
        "inps", [1, BPC * S], bf16, isOutput=False
    )
    # row (snip*128 + i) holds [c*176 + j] -> data[snip, clip(j-5), 0, c*128+i]
    dataT2p = nc.declare_dram_parameter(
        "dataT2p", [NSNIP * 128, EC * PPAD], bf16, isOutput=False
    )
    # diagonal weight matrices: [i, (c*11+k)*128 + j] = w[k, c*128+i] iff i==j
    diagw = nc.declare_dram_parameter(
        "diagw", [128, EC * W * 128], bf16, isOutput=False
    )
    # bf16 output, widened to f32 on the host
    out = nc.declare_dram_parameter("out", [BPC * S, E], bf16, isOutput=True)

    with tile.TileContext(nc) as tc:
        with (
            tc.tile_pool(name="const", bufs=1) as constp,
            tc.tile_pool(name="work", bufs=1) as workp,
            tc.tile_pool(name="ob", bufs=8) as obp,
            tc.tile_pool(name="psA", bufs=2, space="PSUM") as psA,
            tc.tile_pool(name="psB", bufs=2, space="PSUM") as psB,
        ):
            # ---------- tiny constants ----------
            ones1 = constp.tile([1, 128], bf16)
            nc.vector.memset(ones1[:], 1.0)
            warm = constp.tile([128, 512], bf16)
            nc.vector.memset(warm[:], 0.001)

            iota_i = constp.tile([128, 1], i32)
            nc.gpsimd.iota(iota_i[:], [[1, 1]], base=0, channel_multiplier=1)
            iota_f = constp.tile([128, 1], f32)
            nc.vector.tensor_copy(iota_f[:], iota_i[:])
            iota_f_hi = constp.tile([128, 1], f32)
            nc.vector.tensor_scalar_add(iota_f_hi[:], iota_f[:], 128.0)
            iota_m48 = constp.tile([128, 1], f32)
            nc.vector.tensor_scalar_add(iota_m48[:], iota_f[:], -48.0)
            iota_p80 = constp.tile([128, 1], f32)
            nc.vector.tensor_scalar_add(iota_p80[:], iota_f[:], 80.0)

            # ---------- input DMAs (issue ASAP, spread across queues) ----
            # t2 slices go first (they gate conv); the 2.16MB diagw
            # streams behind them, chunk 1 first to match conv's
            # consumption order 1,0,2,3,4,5.
            snipt = workp.tile([1, BPC], i32, tag="snipt")
            nc.sync.dma_start(out=snipt[:], in_=snips_d[:])
            inprt = workp.tile([1, BPC * S], bf16, tag="inprt")
            nc.scalar.dma_start(out=inprt[:], in_=inps_d[:])

            diagb = constp.tile([128, EC * W, 128], bf16)

            def diag_chunk(c, eng):
                eng.dma_start(
                    out=diagb[:, c * W : (c + 1) * W, :],
                    in_=diagw[:, c * W * 128 : (c + 1) * W * 128].rearrange(
                        "p (k j) -> p k j", j=128
                    ),
                )

            snip_v = [
                nc.values_load(
                    snipt[0:1, 0:1],
                    engines=[ET.SP],
                    min_val=0,
                    max_val=NSNIP - 1,
                    skip_runtime_bounds_check=True,
                ),
                nc.values_load(
                    snipt[0:1, 1:2],
                    engines=[ET.Activation],
                    min_val=0,
                    max_val=NSNIP - 1,
                    skip_runtime_bounds_check=True,
                ),
            ]
            t2 = []
            for b, eng in ((0, nc.sync), (1, nc.scalar)):
                t2b = workp.tile([128, EC, PPAD], bf16, tag=f"t2_{b}")
                eng.dma_start(
                    out=t2b[:, :, :],
                    in_=dataT2p[bass.ts(snip_v[b], 128), :].rearrange(
                        "p (c j) -> p c j", j=PPAD
                    ),
                )
                t2.append(t2b)
            diag_chunk(1, nc.sync)
            diag_chunk(0, nc.scalar)
            diag_chunk(2, nc.sync)
            diag_chunk(3, nc.scalar)
            diag_chunk(4, nc.sync)
            diag_chunk(5, nc.scalar)

            # ---------- PE warm-up (ramp the p-state) ------------------
            warm_ps = psB.tile([128, E], f32, tag="go")
            for wi in range(WARM_MMS):
                nc.tensor.matmul(
                    out=warm_ps[:, 0:512],
                    lhsT=warm[:, 0:128],
                    rhs=warm[:, 0:512],
                    start=(wi == 0),
                    stop=(wi == WARM_MMS - 1),
                )
            warm_close = constp.tile([128, 1], f32)
            nc.vector.tensor_copy(warm_close[:], warm_ps[:, 0:1])

            # ---------- input broadcast + one-hots ---------------------
            # inpb[b][p, s] = inputs[b, s] replicated over 128 partitions
            inpb = []
            chunks = [(0, 512), (512, 512), (1024, S - 1024)]
            for b in range(BPC):
                ib = workp.tile([128, S], bf16, tag=f"inpb{b}")
                for ci, (n0, nw) in enumerate(chunks):
                    ps_in = psB.tile([128, E], f32, tag="go")
                    nc.tensor.matmul(
                        out=ps_in[:, :nw],
                        lhsT=ones1[:, :],
                        rhs=inprt[0:1, b * S + n0 : b * S + n0 + nw],
                        start=True,
                        stop=True,
                    )
                    nc.vector.tensor_copy(ib[:, n0 : n0 + nw], ps_in[:, :nw])
                inpb.append(ib)

            # one-hots for the 3 merged C row-blocks:
            # b0 rows live in blocks 0,1 (iota, iota+128);
            # b1 rows live in blocks 1,2 at offsets -48, +80.
            oh = []
            for b, (scA, scB) in enumerate(
                ((iota_f, iota_f_hi), (iota_m48, iota_p80))
            ):
                ohA = workp.tile([128, S], bf16, tag=f"ohA_{b}")
                ohB = workp.tile([128, S], bf16, tag=f"ohB_{b}")
                nc.vector.tensor_scalar(
                    ohA[:], inpb[b][:], scA[:, :1], None, AOT.is_equal
                )
                nc.vector.tensor_scalar(
                    ohB[:], inpb[b][:], scB[:, :1], None, AOT.is_equal
                )
                oh.append((ohA, ohB))

            # merged middle window: global positions 128..271
            # (48 cols of b0's tail, 96 cols of b1's head)
            t2mid = workp.tile([128, EC, 144], bf16, tag="t2mid")
            nc.vector.tensor_copy(t2mid[:, :, 0:48], t2[0][:, :, 128:176])
            nc.vector.tensor_copy(t2mid[:, :, 48:144], t2[1][:, :, 0:96])

            # ---------- conv (3 merged row-blocks) + gather + store ----
            # GPSIMD cannot touch PSUM: drains alternate DVE / ACT only.
            def drain(idx, dst, src):
                if idx % 2 == 0:
                    nc.vector.tensor_copy(dst, src)
                else:
                    nc.scalar.copy(dst, src)

            CORDER = (1, 0, 2, 3, 4, 5)  # matches diag chunk DMA arrival

            def conv_block(idx, src_fn, mw):
                psc = psA.tile([128, E], f32, tag="cv")
                for c in CORDER:
                    for k in range(W):
                        nc.tensor.matmul(
                            out=psc[:mw, c * 128 : (c + 1) * 128],
                            lhsT=src_fn(c, k, mw),
                            rhs=diagb[:, c * W + k, :],
                            start=(k == 0),
                            stop=(k == W - 1),
                        )
                cc = workp.tile([128, E], bf16, tag=f"ccb{idx}")
                drain(idx, cc[:mw, :], psc[:mw, :])
                return cc

            def gather_store(b, ohA, ccA, ohB, ccB):
                for m in range(MTILES):
                    mw = min(128, S - m * 128)
                    pso = psB.tile([128, E], f32, tag="go")
                    for ohx, ccx, st in ((ohA, ccA, True), (ohB, ccB, False)):
                        for n0, nw in ((0, 512), (512, 256)):
                            nc.tensor.matmul(
                                out=pso[:mw, n0 : n0 + nw],
                                lhsT=ohx[:, m * 128 : m * 128 + mw],
                                rhs=ccx[:, n0 : n0 + nw],
                                start=st,
                                stop=not st,
                            )
                    t = b * MTILES + m
                    ob = obp.tile([128, E], bf16, tag="ob")
                    # split drain across DVE + ACT to halve the PSUM
                    # write-after-read latency that paces the PE. Each
                    # half gets its OWN output DMA: a DMA that reads a
                    # region written by two engines can miss one of the
                    # two dependencies (observed as NaN/garbage races).
                    r0 = b * S + m * 128
                    nc.vector.tensor_copy(ob[:mw, 0:384], pso[:mw, 0:384])
                    nc.sync.dma_start(
                        out=out[r0 : r0 + mw, 0:384], in_=ob[:mw, 0:384]
                    )
                    nc.scalar.copy(ob[:mw, 384:E], pso[:mw, 384:E])
                    nc.scalar.dma_start(
                        out=out[r0 : r0 + mw, 384:E], in_=ob[:mw, 384:E]
                    )

            # block 0: b0 rows 0..127; block 1 (t2mid): b0 128..165 +
            # b1 0..79; block 2: b1 rows 80..165
            C0 = conv_block(0, lambda c, k, mw: t2[0][:, c, k : k + mw], 128)
            C1 = conv_block(1, lambda c, k, mw: t2mid[:, c, k : k + mw], 128)
            gather_store(0, oh[0][0], C0, oh[0][1], C1)
            C2 = conv_block(
                2, lambda c, k, mw: t2[1][:, c, 80 + k : 80 + k + mw], 86
            )
            gather_store(1, oh[1][0], C1, oh[1][1], C2)

    nc.finalize()
    return nc


def _get_nc():
    if "nc" not in _cache:
        _cache["nc"] = _build()
    return _cache["nc"]


def _prep_shared(data, w):
    # layout-only host staging (transpose/reshape/edge-pad/dtype-cast)
    import ml_dtypes

    d0 = np.asarray(data, dtype=np.float32)[:, :, 0, :]  # [100, 166, 768]
    pos = np.clip(np.arange(PPAD) - 5, 0, P - 1)
    dp = d0[:, pos, :]  # [100, 176, 768] with clip-pads baked in
    dp = np.transpose(dp, (0, 2, 1))  # [100, 768, 176]
    dp = dp.reshape(NSNIP, EC, 128, PPAD).transpose(0, 2, 1, 3)
    dataT2p = np.ascontiguousarray(
        dp.reshape(NSNIP * 128, EC * PPAD).astype(ml_dtypes.bfloat16)
    )
    wT = np.asarray(w, dtype=np.float32).T  # [768, 11]
    w2 = wT.reshape(EC, 128, W).transpose(1, 0, 2)  # [128, EC, W]
    diagw = np.zeros((128, EC * W, 128), dtype=ml_dtypes.bfloat16)
    ii = np.arange(128)
    diagw[ii, :, ii] = w2.reshape(128, EC * W).astype(ml_dtypes.bfloat16)
    diagw = np.ascontiguousarray(diagw.reshape(128, EC * W * 128))
    return dataT2p, diagw


def kernel(inputs, code_snippet_id, data, w, _trace=False):
    import ml_dtypes
    from concourse.bass_utils import run_bass_kernel_spmd

    nc = _get_nc()
    inputs = np.asarray(inputs, dtype=np.int32)
    code_snippet_id = np.asarray(code_snippet_id, dtype=np.int32)
    dataT2p, diagw = _prep_shared(data, w)

    in_maps = []
    for ci in range(N_CORES):
        b0 = ci * BPC
        in_maps.append(
            {
                "snips": np.ascontiguousarray(
                    code_snippet_id[b0 : b0 + BPC].reshape(1, BPC)
                ),
                "inps": np.ascontiguousarray(
                    inputs[b0 : b0 + BPC]
                    .reshape(1, BPC * S)
                    .astype(ml_dtypes.bfloat16)
                ),
                "dataT2p": dataT2p,
                "diagw": diagw,
            }
        )

    res = run_bass_kernel_spmd(
        nc, in_maps, core_ids=list(range(N_CORES)), trace=_trace
    )
    _cache["last_results"] = res
    out = np.concatenate(
        [
            np.asarray(res.results[i]["out"]).reshape(BPC, S, E)
            for i in range(N_CORES)
        ],
        axis=0,
    ).astype(np.float32)
    return out
